# revision 17
# baseline (speedup 1.0000x reference)
"""Causal self-attention (B=2, S=2048, D=768, H=12) on 8 TRN2 NeuronCores.

Sharding: core c in [0..8) handles batch b = c // 4 and head-group g = c % 4
(3 heads of head_dim 64 each).  Each core returns a partial output [S, D];
the host sums the 4 head-group partials per batch and adds bo.

Per-core dataflow (transposed layouts, no on-device transposes):
  xT [768, 2048]  (host-transposed x[b])
  qk6[128, 4, 3, 512]: chunk-major QKV projection output; m-tile h holds
     head h's qT in partitions 0:64 and kT in partitions 64:128 (host
     interleaves wq/wk columns per head).  qT is also copied to partitions
     64:128 of qdup by a SBUF-SBUF DMA so score matmuls have both operands
     at partition base 64 (the PE requires equal operand bases).
  v_aug[128, 16, 3*65]  v = xT.T @ wv + bv, plus a ones column per head
     (rowsum lands at psum partition 64)
  per head h, per sq-chunk (512): units of TWO sk-tiles sharing one
     [128,1024] psum + one wide Exp; causal diagonal subtiles tri-masked;
     attnout_unnorm.T [65, sq] += v_aug_h(tile).T @ expsT  (row 64 = rowsum)
     norm: f32 reciprocal of rowsum (DVE) -> partition_broadcast (GPSIMD)
     -> one fused multiply (DVE).  No PE involvement.
  out_partial[sq, 768] = aT.T @ wo_slice   (K=3x64)
A warmup matmul stream runs during the input-DMA head so the PE p-state
clock is at max before real work starts (TRN2 halves the PE clock after any
idle gap until ~3us of continuous execution).
Matmul operands are float16; accumulation and softmax stats stay f32.
"""

import math
from contextlib import ExitStack

import numpy as np

B, S, D, H = 2, 2048, 768, 12
HD = D // H          # 64
HPG = 3              # heads per group
G = HPG * HD         # 192 columns per head group
NCORES = 8
KT = D // 128        # 6 k-tiles of the model dim
SQT = S // 128       # 16 seq tiles
NCH = S // 512       # 4 sq chunks of 512
VW = HPG * (HD + 1)  # 195: v columns + ones column per head

_CACHE = {}


def _build_nc():
    import concourse.mybir as mybir
    import concourse.tile as tile
    from concourse import bacc

    f32 = mybir.dt.float32
    f16 = mybir.dt.float16

    nc = bacc.Bacc()

    xT_d = nc.declare_dram_parameter("xT", [128, KT, S], f16, isOutput=False)
    wqk_d = nc.declare_dram_parameter("wqk", [128, KT, 2 * G], f16, isOutput=False)
    wv_d = nc.declare_dram_parameter("wv", [128, KT, G], f16, isOutput=False)
    wo_d = nc.declare_dram_parameter("wo", [64, HPG, D], f16, isOutput=False)
    bqk_d = nc.declare_dram_parameter("bqk", [128, 3], f32, isOutput=False)
    bv_d = nc.declare_dram_parameter("bv", [1, G], f32, isOutput=False)
    tri_d = nc.declare_dram_parameter("tri", [128, 256], f16, isOutput=False)
    out_d = nc.declare_dram_parameter("out", [S, D], f32, isOutput=True)
    wrm_d = nc.declare_dram_parameter("wrm", [1, 8], f32, isOutput=True)

    with tile.TileContext(nc) as tc, ExitStack() as ctx:
        persist = ctx.enter_context(tc.tile_pool(name="persist", bufs=1))
        exps_p = ctx.enter_context(tc.tile_pool(name="exps", bufs=6))
        recip_p = ctx.enter_context(tc.tile_pool(name="recip", bufs=2))
        outs_p = ctx.enter_context(tc.tile_pool(name="outs", bufs=3))
        # PSUM budget (8 banks): mm 2x1 + sc 2x2 + av 2x1 = 8
        mm_ps = ctx.enter_context(tc.tile_pool(name="mmps", bufs=2, space="PSUM"))
        sc_ps = ctx.enter_context(tc.tile_pool(name="scps", bufs=2, space="PSUM"))
        av_ps = ctx.enter_context(tc.tile_pool(name="avps", bufs=2, space="PSUM"))

        # Persistent SBUF tensors
        xT = persist.tile([128, KT, S], f16, tag="xT")
        wqk = persist.tile([128, KT, 2 * G], f16, tag="wqk")
        wv = persist.tile([128, KT, G], f16, tag="wv")
        wo = persist.tile([64, HPG, D], f16, tag="wo")
        bqk = persist.tile([128, 3], f32, tag="bqk")
        bv1 = persist.tile([1, G], f32, tag="bv1")
        bvb = persist.tile([128, G], f32, tag="bvb")
        tri2 = persist.tile([128, 256], f16, tag="tri2")
        warm = persist.tile([128, 512], f16, tag="warm")
        qk6 = persist.tile([128, NCH, 3, 512], f16, tag="qk6")
        qdup = persist.tile([128, NCH, 3, 512], f16, tag="qdup")
        vaug = persist.tile([128, HPG, SQT, HD + 1], f16, tag="vaug")
        aT = persist.tile([64, HPG, S], f16, tag="aT")

        # Warmup: a dependency-free accumulation chain on the PE so the
        # p-state clock ramps to max while the input DMAs land.  A sliver is
        # DMA'd out so the chain cannot be dead-code-eliminated.
        nc.vector.memset(warm[:], 0.25)
        wps = mm_ps.tile([128, 512], f32, tag="mm")
        NW1, NW2 = 6, 6
        for w in range(NW1 + NW2):
            nc.tensor.matmul(
                wps[:] if w < NW1 else wps[:, 0:128],
                warm[:, 0:128],
                warm[:] if w < NW1 else warm[:, 0:128],
                start=(w == 0), stop=(w == NW1 + NW2 - 1),
            )
        wsb = recip_p.tile([1, 8], f32, tag="wsb")
        nc.vector.tensor_copy(wsb[:], wps[0:1, 0:8])
        nc.sync.dma_start(out=wrm_d[:], in_=wsb[:])

        # Input DMAs, in consumption order: chunk-0 m-tile-0 QKV first.
        nc.sync.dma_start(out=bqk[:], in_=bqk_d[:])
        nc.sync.dma_start(out=wqk[:, :, 0:128], in_=wqk_d[:, :, 0:128])
        nc.sync.dma_start(out=wqk[:, :, 128:384], in_=wqk_d[:, :, 128:384])
        nc.sync.dma_start(
            out=xT[:, :, 0:512], in_=xT_d[:, :, 0:512]
        )
        nc.sync.dma_start(out=wv[:], in_=wv_d[:])
        nc.sync.dma_start(out=tri2[:], in_=tri_d[:])
        nc.sync.dma_start(out=bv1[:], in_=bv_d[:])
        for j in range(1, NCH):
            nc.sync.dma_start(
                out=xT[:, :, j * 512 : (j + 1) * 512],
                in_=xT_d[:, :, j * 512 : (j + 1) * 512],
            )
        nc.sync.dma_start(out=wo[:], in_=wo_d[:])

        nc.gpsimd.partition_broadcast(bvb[:], bv1[:])
        nc.gpsimd.memset(vaug[:], 1.0)

        # ---------- emission helpers ----------
        def emit_qk_mtile(j, m):
            jsl = slice(j * 512, (j + 1) * 512)
            ps = mm_ps.tile([128, 512], f32, tag="mm")
            for k in range(KT):
                nc.tensor.matmul(
                    ps[:],
                    wqk[:, k, m * 128 : (m + 1) * 128],
                    xT[:, k, jsl],
                    start=(k == 0),
                    stop=(k == KT - 1),
                )
            # one wide move: rows 0:64 -> qT of head m, 64:128 -> kT
            nc.vector.tensor_scalar_add(
                qk6[:, j, m, :], ps[:], bqk[:, m : m + 1]
            )
            nc.sync.dma_start(
                out=qdup[64:128, j, m, :], in_=qk6[0:64, j, m, :]
            )

        def emit_v_tile(t):
            ps = mm_ps.tile([128, 512], f32, tag="mm")
            for k in range(KT):
                nc.tensor.matmul(
                    ps[:, 0:G],
                    xT[:, k, t * 128 : (t + 1) * 128],
                    wv[:, k, :],
                    start=(k == 0),
                    stop=(k == KT - 1),
                )
            nc.vector.tensor_tensor(
                vaug[:, :, t, 0:HD],
                ps[:, 0:G].rearrange("p (h c) -> p h c", c=HD),
                bvb[:].rearrange("p (h c) -> p h c", c=HD),
                mybir.AluOpType.add,
            )

        def emit_outproj_tile(t):
            ps1 = mm_ps.tile([128, 512], f32, tag="mm")
            ps2 = mm_ps.tile([128, 512], f32, tag="mm")
            for h in range(HPG):
                lt = aT[:, h, t * 128 : (t + 1) * 128]
                nc.tensor.matmul(
                    ps1[:], lt, wo[:, h, 0:512],
                    start=(h == 0), stop=(h == HPG - 1),
                )
                nc.tensor.matmul(
                    ps2[:, 0:256], lt, wo[:, h, 512:768],
                    start=(h == 0), stop=(h == HPG - 1),
                )
            ot = outs_p.tile([128, D], f32, tag="ot")
            nc.vector.tensor_copy(ot[:, 0:512], ps1[:])
            nc.vector.tensor_copy(ot[:, 512:768], ps2[:, 0:256])
            nc.sync.dma_start(out=out_d[t * 128 : (t + 1) * 128, :], in_=ot[:])

        def make_norm(j, h, aps):
            # aT = attnout_unnorm / rowsum: copy the rowsum row to a
            # base-0 SBUF tile (regular DVE op; custom-DVE/walrus mishandle
            # odd PSUM partition bases), partition broadcast on the idle
            # GPSIMD, one DVE divide.
            def norm():
                rs = recip_p.tile([1, 512], f32, tag="rs")
                nc.vector.tensor_copy(rs[:], aps[64:65, :])
                rb = recip_p.tile([64, 512], f32, tag="rb")
                nc.gpsimd.partition_broadcast(rb[:], rs[:])
                rbr = recip_p.tile([64, 512], f32, tag="rbr")
                nc.vector.reciprocal_approx_fast(rbr[:], rb[:])
                nc.vector.tensor_tensor(
                    aT[:, h, j * 512 : (j + 1) * 512],
                    aps[0:64, :],
                    rbr[:],
                    mybir.AluOpType.mult,
                )
            return norm

        # ---------- interleaved emission ----------
        deferred = []  # (due_unit_count, closure)
        unit_no = 0

        def tick(fillers):
            nonlocal unit_no
            unit_no += 1
            while deferred and unit_no >= deferred[0][0]:
                deferred.pop(0)[1]()
            if fillers and unit_no % max(1, tick.spread) == 0:
                fillers.pop(0)()

        # chunk 0 QKV up front
        for m in range(3):
            emit_qk_mtile(0, m)
        for t in range(4):
            emit_v_tile(t)

        for j in range(NCH):
            # Filler schedule: next chunk's QKV early; out-projections pushed
            # late so the long ACT-bound chunk-3 keeps the PE dense.
            fillers = []
            if j + 1 < NCH:
                fillers += [
                    (lambda m=m, jj=j + 1: emit_qk_mtile(jj, m)) for m in range(3)
                ]
                fillers += [
                    (lambda t=t: emit_v_tile(t)) for t in range(4 * j + 4, 4 * j + 8)
                ]
            if j == 2:
                fillers += [(lambda t=t: emit_outproj_tile(t)) for t in range(0, 4)]
            elif j == 3:
                fillers += [(lambda t=t: emit_outproj_tile(t)) for t in range(4, 10)]
            n_units = HPG * (2 * j + 2)
            tick.spread = max(1, -(-n_units // (len(fillers) + 1)))

            for h in range(HPG):
                aps = av_ps.tile([65, 512], f32, tag="av")
                n_av = 4 * j + 4
                av_emitted = 0
                pending = []  # lists of (sk_tile, ex, ex_col0, c0)

                def emit_av(avs, h=h, aps=aps):
                    nonlocal av_emitted
                    for si, ex, exc0, c0 in avs:
                        av_emitted += 1
                        nc.tensor.matmul(
                            aps[:, c0:512],
                            vaug[:, h, si, :],
                            ex[:, exc0 + c0 : exc0 + 512],
                            start=(av_emitted == 1),
                            stop=(av_emitted == n_av),
                        )

                # units: pairs of sk tiles (i0, i1=i0+1) sharing one
                # [128,1024] psum and one wide exp.  Diagonal halves are
                # c0-clipped; the dead zone between them holds unconsumed
                # exp(garbage).
                for i0 in range(0, 4 * j + 4, 2):
                    i1 = i0 + 1
                    c0a = max(0, (i0 - 4 * j) * 128)
                    c0b = max(0, (i1 - 4 * j) * 128)
                    sps = sc_ps.tile([128, 1024], f32, tag="sc")
                    ex = exps_p.tile([128, 1024], f16, tag="ex")
                    for s, (ii, cc) in enumerate(((i0, c0a), (i1, c0b))):
                        nc.tensor.matmul(
                            sps[:, s * 512 + cc : s * 512 + 512],
                            qk6[64:128, ii // 4, h,
                                (ii % 4) * 128 : (ii % 4) * 128 + 128],
                            qdup[64:128, j, h, cc:512],
                            start=True,
                            stop=True,
                        )
                    nc.scalar.activation(
                        ex[:, c0a:1024], sps[:, c0a:1024],
                        mybir.ActivationFunctionType.Exp,
                    )
                    if i1 >= 4 * j:  # diagonal subtiles -> causal tri mask
                        if i0 >= 4 * j:
                            nc.vector.tensor_tensor(
                                ex[:, c0a : c0a + 128],
                                ex[:, c0a : c0a + 128],
                                tri2[:, 0:128],
                                mybir.AluOpType.mult,
                            )
                        nc.vector.tensor_tensor(
                            ex[:, 512 + c0b : 512 + c0b + 128],
                            ex[:, 512 + c0b : 512 + c0b + 128],
                            tri2[:, 128:256],
                            mybir.AluOpType.mult,
                        )
                    pending.append([(i0, ex, 0, c0a), (i1, ex, 512, c0b)])
                    if len(pending) > 2:
                        emit_av(pending.pop(0))
                    tick(fillers)
                for avs in pending:
                    emit_av(avs)
                deferred.append((unit_no + 2, make_norm(j, h, aps)))

            while fillers:
                fillers.pop(0)()

        # tail: last deferred norms + final chunk's out projection
        for _, fn in deferred:
            fn()
        deferred.clear()
        for t in range(10, 4 * NCH):
            emit_outproj_tile(t)

    nc.compile()
    return nc


def _host_inputs(x, wq, bq, wk, bk, wv, bv, wo):
    """Build the 8 per-core input maps (fp16 operands, pre-shuffled layouts)."""
    scale = 1.0 / math.sqrt(HD)
    tri = np.triu(np.ones((128, 128), np.float16))
    tri2 = np.concatenate([tri, tri], axis=1)  # [128, 256]
    in_maps = []
    for c in range(NCORES):
        b, g = divmod(c, 4)
        sl = slice(g * G, (g + 1) * G)
        xT = np.ascontiguousarray(x[b].T).reshape(KT, 128, S).transpose(1, 0, 2)
        # per-head interleave: columns [q_h0 | k_h0 | q_h1 | k_h1 | q_h2 | k_h2]
        wqk = np.concatenate(
            [(wq[:, sl] * scale).reshape(D, HPG, HD), wk[:, sl].reshape(D, HPG, HD)],
            axis=2,
        ).reshape(D, 2 * G)
        wqk = wqk.reshape(KT, 128, 2 * G).transpose(1, 0, 2)
        wvs = wv[:, sl].reshape(KT, 128, G).transpose(1, 0, 2)
        wos = wo[sl, :].reshape(HPG, 64, D).transpose(1, 0, 2)
        # m-tile h biases: partitions 0:64 = bq of head h, 64:128 = bk
        bqk2 = np.concatenate(
            [(bq[sl] * scale).reshape(HPG, HD), bk[sl].reshape(HPG, HD)], axis=1
        ).reshape(HPG, 128).T.astype(np.float32)
        in_maps.append(
            {
                "xT": np.ascontiguousarray(xT).astype(np.float16),
                "wqk": np.ascontiguousarray(wqk).astype(np.float16),
                "wv": np.ascontiguousarray(wvs).astype(np.float16),
                "wo": np.ascontiguousarray(wos).astype(np.float16),
                "bqk": np.ascontiguousarray(bqk2),
                "bv": bv[sl].reshape(1, G).astype(np.float32),
                "tri": tri2,
            }
        )
    return in_maps


TRACE = False
LAST_RESULT = None


def kernel(x, mask, wq, bq, wk, bk, wv, bv, wo, bo):
    global LAST_RESULT
    from concourse.bass_utils import run_bass_kernel_spmd

    x = np.asarray(x, np.float32)
    if "nc" not in _CACHE:
        _CACHE["nc"] = _build_nc()
    nc = _CACHE["nc"]

    in_maps = _host_inputs(
        x,
        np.asarray(wq, np.float32),
        np.asarray(bq, np.float32),
        np.asarray(wk, np.float32),
        np.asarray(bk, np.float32),
        np.asarray(wv, np.float32),
        np.asarray(bv, np.float32),
        np.asarray(wo, np.float32),
    )
    res = run_bass_kernel_spmd(nc, in_maps, list(range(NCORES)), trace=TRACE)
    LAST_RESULT = res
    out = np.zeros((B, S, D), np.float32)
    for c in range(NCORES):
        out[c // 4] += res.results[c]["out"]
    out += np.asarray(bo, np.float32)[None, None, :]
    return out


# revision 18
# speedup vs baseline: 1.0516x; 1.0516x over previous
"""Causal self-attention (B=2, S=2048, D=768, H=12) on 8 TRN2 NeuronCores.

Sharding: core c in [0..8) handles batch b = c // 4 and head-group g = c % 4
(3 heads of head_dim 64 each).  Each core returns a partial output [S, D];
the host sums the 4 head-group partials per batch and adds bo.

Per-core dataflow (transposed layouts, no on-device transposes):
  xT [768, 2048]  (host-transposed x[b])
  qk6[128, 4, 3, 512]: chunk-major QKV projection output; m-tile h holds
     head h's qT in partitions 0:64 and kT in partitions 64:128 (host
     interleaves wq/wk columns per head).  qT is also copied to partitions
     64:128 of qdup by a SBUF-SBUF DMA so score matmuls have both operands
     at partition base 64 (the PE requires equal operand bases).
  v_aug[128, 16, 3*65]  v = xT.T @ wv + bv, plus a ones column per head
     (rowsum lands at psum partition 64)
  per head h, per sq-chunk (512): units of TWO sk-tiles sharing one
     [128,1024] psum + one wide Exp; causal diagonal subtiles tri-masked;
     attnout_unnorm.T [65, sq] += v_aug_h(tile).T @ expsT  (row 64 = rowsum)
     norm: f32 reciprocal of rowsum (DVE) -> partition_broadcast (GPSIMD)
     -> one fused multiply (DVE).  No PE involvement.
  out_partial[sq, 768] = aT.T @ wo_slice   (K=3x64)
A warmup matmul stream runs during the input-DMA head so the PE p-state
clock is at max before real work starts (TRN2 halves the PE clock after any
idle gap until ~3us of continuous execution).
Matmul operands are float16; accumulation and softmax stats stay f32.
"""

import math
from contextlib import ExitStack

import numpy as np

B, S, D, H = 2, 2048, 768, 12
HD = D // H          # 64
HPG = 3              # heads per group
G = HPG * HD         # 192 columns per head group
NCORES = 8
KT = D // 128        # 6 k-tiles of the model dim
SQT = S // 128       # 16 seq tiles
NCH = S // 512       # 4 sq chunks of 512
VW = HPG * (HD + 1)  # 195: v columns + ones column per head

_CACHE = {}


def _build_nc():
    import concourse.mybir as mybir
    import concourse.tile as tile
    from concourse import bacc

    f32 = mybir.dt.float32
    f16 = mybir.dt.float16

    nc = bacc.Bacc()

    xT_d = nc.declare_dram_parameter("xT", [128, NCH, KT, 512], f16, isOutput=False)
    wqk_d = nc.declare_dram_parameter("wqk", [128, 3, KT, 128], f16, isOutput=False)
    wv_d = nc.declare_dram_parameter("wv", [128, KT, G], f16, isOutput=False)
    wo_d = nc.declare_dram_parameter("wo", [64, HPG, D], f16, isOutput=False)
    bqk_d = nc.declare_dram_parameter("bqk", [128, 3], f32, isOutput=False)
    bv_d = nc.declare_dram_parameter("bv", [1, G], f32, isOutput=False)
    tri_d = nc.declare_dram_parameter("tri", [128, 256], f16, isOutput=False)
    out_d = nc.declare_dram_parameter("out", [S, D], f32, isOutput=True)
    wrm_d = nc.declare_dram_parameter("wrm", [1, 8], f32, isOutput=True)

    with tile.TileContext(nc) as tc, ExitStack() as ctx:
        persist = ctx.enter_context(tc.tile_pool(name="persist", bufs=1))
        exps_p = ctx.enter_context(tc.tile_pool(name="exps", bufs=6))
        recip_p = ctx.enter_context(tc.tile_pool(name="recip", bufs=2))
        outs_p = ctx.enter_context(tc.tile_pool(name="outs", bufs=3))
        # PSUM budget (8 banks): mm 2x1 + sc 2x2 + av 2x1 = 8
        mm_ps = ctx.enter_context(tc.tile_pool(name="mmps", bufs=2, space="PSUM"))
        sc_ps = ctx.enter_context(tc.tile_pool(name="scps", bufs=2, space="PSUM"))
        av_ps = ctx.enter_context(tc.tile_pool(name="avps", bufs=2, space="PSUM"))

        # Persistent SBUF tensors
        xT = persist.tile([128, NCH, KT, 512], f16, tag="xT")
        wqk = persist.tile([128, 3, KT, 128], f16, tag="wqk")
        wv = persist.tile([128, KT, G], f16, tag="wv")
        wo = persist.tile([64, HPG, D], f16, tag="wo")
        bqk = persist.tile([128, 3], f32, tag="bqk")
        bv1 = persist.tile([1, G], f32, tag="bv1")
        bvb = persist.tile([128, G], f32, tag="bvb")
        tri2 = persist.tile([128, 256], f16, tag="tri2")
        warm = persist.tile([128, 512], f16, tag="warm")
        qk6 = persist.tile([128, NCH, 3, 512], f16, tag="qk6")
        qdup = persist.tile([128, NCH, 3, 512], f16, tag="qdup")
        vaug = persist.tile([128, HPG, SQT, HD + 1], f16, tag="vaug")
        aT = persist.tile([64, HPG, S], f16, tag="aT")

        # Warmup: a dependency-free accumulation chain on the PE so the
        # p-state clock ramps to max while the input DMAs land.  A sliver is
        # DMA'd out so the chain cannot be dead-code-eliminated.
        nc.gpsimd.memset(warm[:], 0.25)
        wps = mm_ps.tile([128, 512], f32, tag="mm")
        NW1, NW2 = 6, 8
        for w in range(NW1 + NW2):
            nc.tensor.matmul(
                wps[:] if w < NW1 else wps[:, 0:128],
                warm[:, 0:128],
                warm[:] if w < NW1 else warm[:, 0:128],
                start=(w == 0), stop=(w == NW1 + NW2 - 1),
            )
        wsb = recip_p.tile([1, 8], f32, tag="wsb")
        nc.vector.tensor_copy(wsb[:], wps[0:1, 0:8])
        nc.sync.dma_start(out=wrm_d[:], in_=wsb[:])

        # Input DMAs, in consumption order: chunk-0 m-tile-0 QKV first.
        nc.sync.dma_start(out=bqk[:], in_=bqk_d[:])
        nc.sync.dma_start(out=wqk[:, 0], in_=wqk_d[:, 0])
        nc.sync.dma_start(out=wqk[:, 1:3], in_=wqk_d[:, 1:3])
        nc.sync.dma_start(out=xT[:, 0], in_=xT_d[:, 0])
        nc.sync.dma_start(out=wv[:], in_=wv_d[:])
        nc.sync.dma_start(out=tri2[:], in_=tri_d[:])
        nc.sync.dma_start(out=bv1[:], in_=bv_d[:])
        for j in range(1, NCH):
            nc.sync.dma_start(out=xT[:, j], in_=xT_d[:, j])
        nc.sync.dma_start(out=wo[:], in_=wo_d[:])

        nc.gpsimd.partition_broadcast(bvb[:], bv1[:])
        nc.gpsimd.memset(vaug[:], 1.0)

        # ---------- emission helpers ----------
        def emit_qk_mtile(j, m):
            ps = mm_ps.tile([128, 512], f32, tag="mm")
            for k in range(KT):
                nc.tensor.matmul(
                    ps[:],
                    wqk[:, m, k, :],
                    xT[:, j, k, :],
                    start=(k == 0),
                    stop=(k == KT - 1),
                )
            # one wide move: rows 0:64 -> qT of head m, 64:128 -> kT
            nc.vector.tensor_scalar_add(
                qk6[:, j, m, :], ps[:], bqk[:, m : m + 1]
            )
            nc.sync.dma_start(
                out=qdup[64:128, j, m, :], in_=qk6[0:64, j, m, :]
            )

        def emit_v_tile(t):
            ps = mm_ps.tile([128, 512], f32, tag="mm")
            for k in range(KT):
                nc.tensor.matmul(
                    ps[:, 0:G],
                    xT[:, t // 4, k, (t % 4) * 128 : (t % 4) * 128 + 128],
                    wv[:, k, :],
                    start=(k == 0),
                    stop=(k == KT - 1),
                )
            nc.vector.tensor_tensor(
                vaug[:, :, t, 0:HD],
                ps[:, 0:G].rearrange("p (h c) -> p h c", c=HD),
                bvb[:].rearrange("p (h c) -> p h c", c=HD),
                mybir.AluOpType.add,
            )

        def emit_outproj_tile(t, tail=False):
            if tail:
                # score psum banks are idle in the tail; one wide bank and a
                # single copy avoids the mm-bank/DVE recycle stall
                ps = sc_ps.tile([128, 1024], f32, tag="sc")
                ps1, ps2 = ps[:, 0:512], ps[:, 512:1024]
            else:
                ps1 = mm_ps.tile([128, 512], f32, tag="mm")
                ps2 = mm_ps.tile([128, 512], f32, tag="mm")
            for h in range(HPG):
                lt = aT[:, h, t * 128 : (t + 1) * 128]
                nc.tensor.matmul(
                    ps1[:, 0:512], lt, wo[:, h, 0:512],
                    start=(h == 0), stop=(h == HPG - 1),
                )
                nc.tensor.matmul(
                    ps2[:, 0:256], lt, wo[:, h, 512:768],
                    start=(h == 0), stop=(h == HPG - 1),
                )
            ot = outs_p.tile([128, D], f32, tag="ot")
            if tail:
                nc.vector.tensor_copy(
                    ot[:].rearrange("p (a b) -> p a b", a=2),
                    ps[:, 0:768].rearrange("p (a b) -> p a b", a=2),
                )
            else:
                nc.vector.tensor_copy(ot[:, 0:512], ps1[:])
                nc.vector.tensor_copy(ot[:, 512:768], ps2[:, 0:256])
            nc.sync.dma_start(out=out_d[t * 128 : (t + 1) * 128, :], in_=ot[:])

        def make_norm(j, h, aps):
            # aT = attnout_unnorm / rowsum: copy the rowsum row to a
            # base-0 SBUF tile (regular DVE op; custom-DVE/walrus mishandle
            # odd PSUM partition bases), partition broadcast on the idle
            # GPSIMD, one DVE divide.
            def norm():
                rs = recip_p.tile([1, 512], f32, tag="rs")
                nc.vector.tensor_copy(rs[:], aps[64:65, :])
                rb = recip_p.tile([64, 512], f32, tag="rb")
                nc.gpsimd.partition_broadcast(rb[:], rs[:])
                rbr = recip_p.tile([64, 512], f32, tag="rbr")
                nc.vector.reciprocal_approx_fast(rbr[:], rb[:])
                nc.vector.tensor_tensor(
                    aT[:, h, j * 512 : (j + 1) * 512],
                    aps[0:64, :],
                    rbr[:],
                    mybir.AluOpType.mult,
                )
            return norm

        # ---------- interleaved emission ----------
        deferred = []  # (due_unit_count, closure)
        unit_no = 0

        def tick(fillers):
            nonlocal unit_no
            unit_no += 1
            while deferred and unit_no >= deferred[0][0]:
                deferred.pop(0)[1]()
            if fillers and unit_no % max(1, tick.spread) == 0:
                fillers.pop(0)()

        # chunk 0 QKV up front
        for m in range(3):
            emit_qk_mtile(0, m)
        for t in range(4):
            emit_v_tile(t)

        for j in range(NCH):
            # Filler schedule: next chunk's QKV early; out-projections pushed
            # late so the long ACT-bound chunk-3 keeps the PE dense.
            fillers = []
            if j + 1 < NCH:
                fillers += [
                    (lambda m=m, jj=j + 1: emit_qk_mtile(jj, m)) for m in range(3)
                ]
                fillers += [
                    (lambda t=t: emit_v_tile(t)) for t in range(4 * j + 4, 4 * j + 8)
                ]
            if j == 2:
                fillers += [(lambda t=t: emit_outproj_tile(t)) for t in range(0, 4)]
            elif j == 3:
                fillers += [(lambda t=t: emit_outproj_tile(t)) for t in range(4, 10)]
            n_units = HPG * (2 * j + 2)
            tick.spread = max(1, -(-n_units // (len(fillers) + 1)))

            for h in range(HPG):
                aps = av_ps.tile([65, 512], f32, tag="av")
                n_av = 4 * j + 4
                av_emitted = 0
                pending = []  # lists of (sk_tile, ex, ex_col0, c0)

                def emit_av(avs, h=h, aps=aps):
                    nonlocal av_emitted
                    for si, ex, exc0, c0 in avs:
                        av_emitted += 1
                        nc.tensor.matmul(
                            aps[:, c0:512],
                            vaug[:, h, si, :],
                            ex[:, exc0 + c0 : exc0 + 512],
                            start=(av_emitted == 1),
                            stop=(av_emitted == n_av),
                        )

                # units: pairs of sk tiles (i0, i1=i0+1) sharing one
                # [128,1024] psum and one wide exp.  Diagonal halves are
                # c0-clipped; the dead zone between them holds unconsumed
                # exp(garbage).
                for i0 in range(0, 4 * j + 4, 2):
                    i1 = i0 + 1
                    c0a = max(0, (i0 - 4 * j) * 128)
                    c0b = max(0, (i1 - 4 * j) * 128)
                    sps = sc_ps.tile([128, 1024], f32, tag="sc")
                    ex = exps_p.tile([128, 1024], f16, tag="ex")
                    for s, (ii, cc) in enumerate(((i0, c0a), (i1, c0b))):
                        nc.tensor.matmul(
                            sps[:, s * 512 + cc : s * 512 + 512],
                            qk6[64:128, ii // 4, h,
                                (ii % 4) * 128 : (ii % 4) * 128 + 128],
                            qdup[64:128, j, h, cc:512],
                            start=True,
                            stop=True,
                        )
                    nc.scalar.activation(
                        ex[:, c0a:1024], sps[:, c0a:1024],
                        mybir.ActivationFunctionType.Exp,
                    )
                    if i1 >= 4 * j:  # diagonal subtiles -> causal tri mask
                        if i0 >= 4 * j:
                            nc.vector.tensor_tensor(
                                ex[:, c0a : c0a + 128],
                                ex[:, c0a : c0a + 128],
                                tri2[:, 0:128],
                                mybir.AluOpType.mult,
                            )
                        nc.vector.tensor_tensor(
                            ex[:, 512 + c0b : 512 + c0b + 128],
                            ex[:, 512 + c0b : 512 + c0b + 128],
                            tri2[:, 128:256],
                            mybir.AluOpType.mult,
                        )
                    pending.append([(i0, ex, 0, c0a), (i1, ex, 512, c0b)])
                    if len(pending) > 2:
                        emit_av(pending.pop(0))
                    tick(fillers)
                for avs in pending:
                    emit_av(avs)
                deferred.append((unit_no + 2, make_norm(j, h, aps)))

            while fillers:
                fillers.pop(0)()

        # tail: last deferred norms + final chunk's out projection
        for _, fn in deferred:
            fn()
        deferred.clear()
        for t in range(10, 4 * NCH):
            emit_outproj_tile(t, tail=True)

    nc.compile()
    return nc


def _host_inputs(x, wq, bq, wk, bk, wv, bv, wo):
    """Build the 8 per-core input maps (fp16 operands, pre-shuffled layouts)."""
    scale = 1.0 / math.sqrt(HD)
    tri = np.triu(np.ones((128, 128), np.float16))
    tri2 = np.concatenate([tri, tri], axis=1)  # [128, 256]
    in_maps = []
    for c in range(NCORES):
        b, g = divmod(c, 4)
        sl = slice(g * G, (g + 1) * G)
        xT = np.ascontiguousarray(x[b].T).reshape(KT, 128, NCH, 512)
        xT = xT.transpose(1, 2, 0, 3)  # [128, NCH, KT, 512]
        # per-head interleave: columns [q_h0 | k_h0 | q_h1 | k_h1 | q_h2 | k_h2]
        wqk = np.concatenate(
            [(wq[:, sl] * scale).reshape(D, HPG, HD), wk[:, sl].reshape(D, HPG, HD)],
            axis=2,
        )  # [D, HPG, 128]
        wqk = wqk.reshape(KT, 128, HPG, 128).transpose(1, 2, 0, 3)  # [128,3,KT,128]
        wvs = wv[:, sl].reshape(KT, 128, G).transpose(1, 0, 2)
        wos = wo[sl, :].reshape(HPG, 64, D).transpose(1, 0, 2)
        # m-tile h biases: partitions 0:64 = bq of head h, 64:128 = bk
        bqk2 = np.concatenate(
            [(bq[sl] * scale).reshape(HPG, HD), bk[sl].reshape(HPG, HD)], axis=1
        ).reshape(HPG, 128).T.astype(np.float32)
        in_maps.append(
            {
                "xT": np.ascontiguousarray(xT).astype(np.float16),
                "wqk": np.ascontiguousarray(wqk).astype(np.float16),
                "wv": np.ascontiguousarray(wvs).astype(np.float16),
                "wo": np.ascontiguousarray(wos).astype(np.float16),
                "bqk": np.ascontiguousarray(bqk2),
                "bv": bv[sl].reshape(1, G).astype(np.float32),
                "tri": tri2,
            }
        )
    return in_maps


TRACE = False
LAST_RESULT = None


def kernel(x, mask, wq, bq, wk, bk, wv, bv, wo, bo):
    global LAST_RESULT
    from concourse.bass_utils import run_bass_kernel_spmd

    x = np.asarray(x, np.float32)
    if "nc" not in _CACHE:
        _CACHE["nc"] = _build_nc()
    nc = _CACHE["nc"]

    in_maps = _host_inputs(
        x,
        np.asarray(wq, np.float32),
        np.asarray(bq, np.float32),
        np.asarray(wk, np.float32),
        np.asarray(bk, np.float32),
        np.asarray(wv, np.float32),
        np.asarray(bv, np.float32),
        np.asarray(wo, np.float32),
    )
    res = run_bass_kernel_spmd(nc, in_maps, list(range(NCORES)), trace=TRACE)
    LAST_RESULT = res
    out = np.zeros((B, S, D), np.float32)
    for c in range(NCORES):
        out[c // 4] += res.results[c]["out"]
    out += np.asarray(bo, np.float32)[None, None, :]
    return out
